# revision 11
# baseline (speedup 1.0000x reference)
"""Trainium2 Bass kernel for nn_DNN_65944927863183 (dense_mlp).

Network (per row of input [B=262144, F=768]):
    4 blocks of  x = LN(x; g,b) @ W.T + c ; ELU between blocks (not after last)
    sizes 768 -> 512 -> 256 -> 128 -> 1, output squeezed to [B].

Strategy
--------
Pure data parallel over 8 NeuronCores: each core gets B/8 = 32768 contiguous
rows, weights replicated. Inside a core, rows are processed in 128-row tiles
(rows on SBUF partitions), 16 tiles per "group" for batching small ops.

Math tricks (all exact up to float rounding):
 * LN affine (g, b) and linear bias c are folded into the weights on the host:
   y = LNhat(x) @ (W*g).T + (b@W.T + c), LNhat = (x-m)/sqrt(var+eps).
 * LN is invariant to per-row affine of its input. So each layer stores
   x' = q * (ELU(z) + 1)   (q = std of this layer's input, per row)
   instead of ELU(z); the next LN removes the per-row scale/shift exactly
   (the 1e-5 epsilon is scaled by q^2 in later layers; |error| ~ 1e-4 rel).
 * Normalization is folded into the matmul: with stored input x'' (stats m,
   q = sqrt(var+eps), r = 1/q), the pre-activation in PSUM is
       P = x''@Wg.T + m*(-s) + q*c     (s_j = sum_f Wg[j,f]),
   via one K=2 "correction matmul" with lhsT rows (m, q); then z = r*P.
 * ELU+1, scaled by q:  q*(ELU(z)+1) = min( exp(r*P + ln q), max(P+q, q) )
   -> one ScalarE Exp (psum->sbuf), one VectorE tensor_scalar (add-then-max),
   one VectorE scalar_tensor_tensor min (with a free running sum for the next
   layer's mean via accum_out).
 * r and q come from v (variance) with only the Exp/Ln activation table set:
   lv = ln(v+eps); r = exp(-lv/2); q = exp(+lv/2); ln q = lv/2.
 * Matmuls contract over SBUF partitions, so activations are transposed
   per layer with regular matmuls against an fp16 identity (full PE speed,
   keeps the PE HAM-warm); layer-0 transposed tiles are loaded straight from
   HBM with the DMA xbar transpose (bf16).

Everything on the PE is fp16 (inputs cast host-side), accumulation fp32.
"""

import os
import sys

import numpy as np

sys.path.insert(0, "/opt/trn_rl_repo")

import ml_dtypes

B, F = 262144, 768
N_CORES = 8
ROWS_PER_CORE = B // N_CORES  # 32768
DINS = [768, 512, 256, 128]
DOUTS = [512, 256, 128, 1]
NCH = [d // 128 for d in DINS]  # k-chunks per layer: 6,4,2,1
TILE = 128
EPS = 1e-5

F16 = np.float16

_BUILT = {}
LAST_RESULT = None


def _build(nrows, G):
    """Build + trace the single-core Bass program for `nrows` rows."""
    import concourse.bacc as bacc
    import concourse.tile as tile
    from concourse import mybir

    dt = mybir.dt
    ALU = mybir.AluOpType
    ACTF = mybir.ActivationFunctionType

    ntiles = nrows // TILE
    ngroups = ntiles // G
    assert ntiles % G == 0 and nrows % TILE == 0

    nc = bacc.Bacc("TRN2", target_bir_lowering=False)

    x_d = nc.dram_tensor("x", (nrows, F), dt.float16, kind="ExternalInput")
    w_d = [
        nc.dram_tensor(f"w{i}", (DINS[i], DOUTS[i]), dt.float16, kind="ExternalInput")
        for i in range(4)
    ]
    sc_d = [
        nc.dram_tensor(
            f"scm{i}", (G, 2 * G, DOUTS[i]), dt.float16, kind="ExternalInput"
        )
        for i in range(4)
    ]
    idb_d = nc.dram_tensor("idb", (128, 128), dt.float16, kind="ExternalInput")
    idf_d = nc.dram_tensor("idf", (128, 128), dt.float32, kind="ExternalInput")
    y_d = nc.dram_tensor("y", (nrows,), dt.float32, kind="ExternalOutput")

    xap = x_d[:]
    y_view = y_d[:].rearrange("(a b) -> a b", b=TILE)  # [ntiles, 128]

    GR = G * TILE  # rows per group

    with tile.TileContext(nc) as tc:
        with (
            tc.tile_pool(name="consts", bufs=1) as consts,
            tc.tile_pool(name="xrow", bufs=2 * G) as xrow_p,
            tc.tile_pool(name="xt0", bufs=2) as xt0_p,
            tc.tile_pool(name="gstats", bufs=2) as gst_p,
            tc.tile_pool(name="work", bufs=4) as work_p,
            tc.tile_pool(name="xt", bufs=2 * G + 2) as xt_p,
            tc.tile_pool(name="ybuf", bufs=1) as ybuf_p,
            tc.tile_pool(name="mm_ps", bufs=3, space="PSUM") as mm_ps,
            tc.tile_pool(name="tp_ps", bufs=2, space="PSUM") as tp_ps,
            tc.tile_pool(name="y3_ps", bufs=2, space="PSUM") as y3_ps,
            tc.tile_pool(name="mq_ps", bufs=1, space="PSUM") as mq_ps,
        ):
            # ---- constants ----
            wts = []
            for i in range(4):
                w = consts.tile([128, NCH[i], DOUTS[i]], dt.float16, tag=f"wt{i}")
                nc.sync.dma_start(
                    out=w, in_=w_d[i][:].rearrange("(n p) d -> p n d", p=128)
                )
                wts.append(w)
            scs = []
            for i in range(4):
                s = consts.tile([2 * G, G, DOUTS[i]], dt.float16, tag=f"scm{i}")
                nc.sync.dma_start(out=s, in_=sc_d[i][:].rearrange("t k d -> k t d"))
                scs.append(s)
            idb = consts.tile([128, 128], dt.float16, tag="idb")
            nc.sync.dma_start(out=idb, in_=idb_d[:])
            idf = consts.tile([128, 128], dt.float32, tag="idf")
            nc.sync.dma_start(out=idf, in_=idf_d[:])
            eps_t = consts.tile([128, 1], dt.float32, tag="eps")
            nc.vector.memset(eps_t, EPS)

            y_sb = ybuf_p.tile([128, ntiles], dt.float32, tag="ysb")

            for g in range(ngroups):
                r0 = g * GR
                # ---------- loads ----------
                xrows = []
                for t in range(G):
                    xr = xrow_p.tile([128, F], dt.float16, tag="xr")
                    nc.sync.dma_start(
                        out=xr, in_=xap[r0 + t * TILE : r0 + (t + 1) * TILE, :]
                    )
                    xrows.append(xr)
                xt0 = xt0_p.tile([128, NCH[0], GR], dt.float16, tag="xt0")
                for k in range(NCH[0]):
                    nc.sync.dma_start(
                        out=xt0[:, k, :],
                        in_=xap[r0 : r0 + GR, k * 128 : (k + 1) * 128],
                        transpose=True,
                    )

                # ---------- per-group stats state ----------
                mq_l, r_l, q_l, lnq_l, mqT_l = {}, {}, {}, {}, {}

                def group_stats(l, m_src, v_src):
                    """m_src, v_src: [128, G] f32 APs (mean and variance)."""
                    lv = gst_p.tile([128, G], dt.float32, tag=f"lv{l}")
                    nc.scalar.activation(out=lv, in_=v_src, func=ACTF.Ln, bias=eps_t)
                    r_ = gst_p.tile([128, G], dt.float32, tag=f"r{l}")
                    nc.scalar.activation(out=r_, in_=lv, func=ACTF.Exp, scale=-0.5)
                    q_ = gst_p.tile([128, G], dt.float32, tag=f"q{l}")
                    nc.scalar.activation(out=q_, in_=lv, func=ACTF.Exp, scale=0.5)
                    lnq = gst_p.tile([128, G], dt.float32, tag=f"lnq{l}")
                    nc.vector.tensor_scalar_mul(out=lnq, in0=lv, scalar1=0.5)
                    mq = gst_p.tile([128, G, 2], dt.float16, tag=f"mq{l}")
                    nc.vector.tensor_scalar_mul(out=mq[:, :, 0], in0=m_src, scalar1=1.0)
                    nc.vector.tensor_scalar_mul(out=mq[:, :, 1], in0=q_, scalar1=1.0)
                    # transpose mq -> [2G, 128] for corr-matmul lhsT
                    ps = mq_ps.tile([2 * G, 128], dt.float32, tag="mqps")
                    nc.tensor.matmul(
                        ps, mq[:].rearrange("p a b -> p (a b)"), idb,
                        start=True, stop=True,
                    )
                    mqT = gst_p.tile([2 * G, 128], dt.float16, tag=f"mqT{l}")
                    nc.vector.tensor_copy(out=mqT, in_=ps)
                    mq_l[l], r_l[l], q_l[l], lnq_l[l], mqT_l[l] = mq, r_, q_, lnq, mqT

                # ---------- layer 0 stats (bn_stats over row-major x) ----------
                mv0 = gst_p.tile([128, G, 2], dt.float32, tag="mv0")
                for t in range(G):
                    st = work_p.tile([128, 2, 6], dt.float32, tag="bnst")
                    xr3 = xrows[t][:].rearrange("p (c d) -> p c d", c=2)
                    nc.vector.bn_stats(out=st[:, 0, :], in_=xr3[:, 0, :])
                    nc.vector.bn_stats(out=st[:, 1, :], in_=xr3[:, 1, :])
                    nc.vector.bn_aggr(out=mv0[:, t, :], in_=st)
                group_stats(0, mv0[:, :, 0], mv0[:, :, 1])

                # running sums for layers 1..3 stats
                msum, ssq = {}, {}
                for l in (1, 2, 3):
                    msum[l] = gst_p.tile([128, G], dt.float32, name=f"msum{l}", tag=f"msum{l}")
                    ssq[l] = gst_p.tile([128, G], dt.float32, name=f"ssq{l}", tag=f"ssq{l}")

                xts = {0: [xt0[:, :, t * TILE : (t + 1) * TILE] for t in range(G)]}

                # ---------- layers ----------
                for l in range(4):
                    din, dout, nch = DINS[l], DOUTS[l], NCH[l]
                    if l > 0:
                        ms = gst_p.tile([128, G], dt.float32, tag=f"ms{l}")
                        nc.vector.tensor_scalar_mul(
                            out=ms, in0=msum[l], scalar1=1.0 / din
                        )
                        m2 = gst_p.tile([128, G], dt.float32, tag=f"m2{l}")
                        nc.vector.tensor_mul(m2, ms, ms)
                        v = gst_p.tile([128, G], dt.float32, tag=f"v{l}")
                        nc.vector.scalar_tensor_tensor(
                            out=v,
                            in0=ssq[l],
                            scalar=1.0 / din,
                            in1=m2,
                            op0=ALU.mult,
                            op1=ALU.subtract,
                        )
                        group_stats(l, ms, v)

                    mqT = mqT_l[l]
                    r_, q_, lnq = r_l[l], q_l[l], lnq_l[l]

                    if l == 3:
                        yps = y3_ps.tile([128, G], dt.float32, tag="y3")

                    for t in range(G):
                        xT = xts[l][t]  # [128, nch, 128] view
                        # ---- matmul: P = corr + x'' @ Wg.T ----
                        if l < 3:
                            ps = mm_ps.tile([128, dout], dt.float32, tag="mm")
                        else:
                            ps = yps[:, t : t + 1]
                        nc.tensor.matmul(
                            ps, mqT, scs[l][:, t, :],
                            start=True, stop=False,
                        )
                        for k in range(nch):
                            nc.tensor.matmul(
                                ps, xT[:, k, :], wts[l][:, k, :],
                                start=False, stop=(k == nch - 1),
                            )
                        if l == 3:
                            continue

                        # ---- ELU(+1, q-scaled) ----
                        rt = r_[:, t : t + 1]
                        qt = q_[:, t : t + 1]
                        lnqt = lnq[:, t : t + 1]
                        E = work_p.tile([128, dout], dt.float16, tag=f"E{l}")
                        nc.scalar.activation(
                            out=E, in_=ps, func=ACTF.Exp, scale=rt, bias=lnqt
                        )
                        T = work_p.tile([128, dout], dt.float16, tag=f"T{l}")
                        nc.vector.tensor_scalar(
                            out=T,
                            in0=ps,
                            scalar1=qt,
                            scalar2=qt,
                            op0=ALU.add,
                            op1=ALU.max,
                        )
                        xn = work_p.tile([128, dout], dt.float16, tag=f"xn{l}")
                        nc.vector.scalar_tensor_tensor(
                            out=xn,
                            in0=T,
                            scalar=0.0,
                            in1=E,
                            op0=ALU.add,
                            op1=ALU.min,
                            accum_out=msum[l + 1][:, t : t + 1],
                        )
                        # sum of squares for next layer's variance
                        sq = work_p.tile([128, dout], dt.float16, tag=f"sq{l}")
                        nc.scalar.activation(
                            out=sq,
                            in_=xn,
                            func=ACTF.Square,
                            accum_out=ssq[l + 1][:, t : t + 1],
                        )
                        # ---- transpose xn for next layer ----
                        ncho = dout // 128
                        tps = tp_ps.tile([128, ncho, 128], dt.float32, tag="tp")
                        for k in range(ncho):
                            nc.tensor.matmul(
                                tps[:, k, :],
                                xn[:, k * 128 : (k + 1) * 128],
                                idb,
                                start=True,
                                stop=True,
                            )
                        xTn = xt_p.tile([128, ncho, 128], dt.float16, tag=f"xT{l + 1}")
                        nc.vector.tensor_copy(out=xTn, in_=tps)
                        xts.setdefault(l + 1, {})[t] = xTn

                    # ---- layer 3 epilogue: y = r3 * P3 for the whole group ----
                    if l == 3:
                        nc.vector.tensor_mul(y_sb[:, g * G : (g + 1) * G], yps, r_)

            # ---------- output: transpose y_sb [128, ntiles] and store ----------
            nfull = ntiles // 128
            for c in range(nfull):
                ps = tp_ps.tile([128, 128], dt.float32, tag="tp")
                nc.tensor.transpose(ps, y_sb[:, c * 128 : (c + 1) * 128], idf)
                yT = work_p.tile([128, 128], dt.float32, tag="yTs")
                nc.vector.tensor_copy(out=yT, in_=ps)
                nc.sync.dma_start(out=y_view[c * 128 : (c + 1) * 128, :], in_=yT)
            for t in range(nfull * 128, ntiles):
                # small-row builds (sim): strided DMA per tile, slow but correct
                nc.sync.dma_start(out=y_view[t, :], in_=y_sb[:, t])

    nc.compile()
    return nc


def _get_program(nrows, G):
    key = (nrows, G)
    if key not in _BUILT:
        _BUILT[key] = _build(nrows, G)
    return _BUILT[key]


def _fold_weights(inputs, G=8):
    """Host-side folding of LN affine + bias into matmul weights."""
    consts = {}
    for i in range(4):
        g = np.asarray(inputs[f"g{i}"], np.float32)
        b = np.asarray(inputs[f"b{i}"], np.float32)
        W = np.asarray(inputs[f"W{i}"], np.float32)  # [dout, din]
        c = np.asarray(inputs[f"c{i}"], np.float32)
        dout, din = W.shape
        Wg = W * g[None, :]
        c_eff = c + b @ W.T
        s = Wg.sum(axis=1)
        consts[f"w{i}"] = np.ascontiguousarray(Wg.T).astype(F16)  # [din, dout]
        scm = np.zeros((G, 2 * G, dout), np.float32)
        for t in range(G):
            scm[t, 2 * t, :] = -s
            scm[t, 2 * t + 1, :] = c_eff
        consts[f"scm{i}"] = scm.astype(F16)  # [G, 2G, dout]
    consts["idb"] = np.eye(128, dtype=F16)
    consts["idf"] = np.eye(128, dtype=np.float32)
    return consts


def _install_ntff_shim():
    """bass_utils' trace path needs antenv.axon_hooks, absent in this image.
    Recreate it with the ctypes hook from trn_agent_boot (same mechanism)."""
    import types

    try:
        from antenv.axon_hooks import get_axon_ntff_profile_hook  # noqa: F401

        return
    except ImportError:
        pass
    try:
        import antenv
        from trn_agent_boot.trn_boot import _ntff_profile_via_ctypes

        hook = _ntff_profile_via_ctypes("/opt/axon/libaxon_pjrt.so")
        mod = types.ModuleType("antenv.axon_hooks")
        mod._hook = hook
        mod.get_axon_ntff_profile_hook = lambda: mod._hook
        mod.set_axon_ntff_profile_hook = lambda h: setattr(mod, "_hook", h)
        sys.modules["antenv.axon_hooks"] = mod
        antenv.axon_hooks = mod
    except Exception as e:  # tracing is best-effort
        print(f"ntff shim failed: {e}")


def kernel(**inputs):
    from concourse.bass_utils import run_bass_kernel_spmd

    x = np.asarray(inputs["input"])
    assert x.shape == (B, F)
    consts = _fold_weights(inputs)
    x_bf = x.astype(F16)

    nc = _get_program(ROWS_PER_CORE, 8)
    in_maps = []
    for cid in range(N_CORES):
        shard = x_bf[cid * ROWS_PER_CORE : (cid + 1) * ROWS_PER_CORE]
        in_maps.append({"x": np.ascontiguousarray(shard), **consts})
    trace = bool(os.environ.get("KERNEL_TRACE"))
    if trace:
        _install_ntff_shim()
    res = run_bass_kernel_spmd(
        nc, in_maps, core_ids=list(range(N_CORES)), trace=trace
    )
    global LAST_RESULT
    LAST_RESULT = res
    out = np.concatenate([np.asarray(res.results[c]["y"]) for c in range(N_CORES)])
    return out.astype(np.float32)


# revision 12
# speedup vs baseline: 1.0683x; 1.0683x over previous
"""Trainium2 Bass kernel for nn_DNN_65944927863183 (dense_mlp).

Network (per row of input [B=262144, F=768]):
    4 blocks of  x = LN(x; g,b) @ W.T + c ; ELU between blocks (not after last)
    sizes 768 -> 512 -> 256 -> 128 -> 1, output squeezed to [B].

Strategy
--------
Pure data parallel over 8 NeuronCores: each core gets B/8 = 32768 contiguous
rows, weights replicated. Inside a core, rows are processed in 128-row tiles
(rows on SBUF partitions), 16 tiles per "group" for batching small ops.

Math tricks (all exact up to float rounding):
 * LN affine (g, b) and linear bias c are folded into the weights on the host:
   y = LNhat(x) @ (W*g).T + (b@W.T + c), LNhat = (x-m)/sqrt(var+eps).
 * LN is invariant to per-row affine of its input. So each layer stores
   x' = q * (ELU(z) + 1)   (q = std of this layer's input, per row)
   instead of ELU(z); the next LN removes the per-row scale/shift exactly
   (the 1e-5 epsilon is scaled by q^2 in later layers; |error| ~ 1e-4 rel).
 * Normalization is folded into the matmul: with stored input x'' (stats m,
   q = sqrt(var+eps), r = 1/q), the pre-activation in PSUM is
       P = x''@Wg.T + m*(-s) + q*c     (s_j = sum_f Wg[j,f]),
   via one K=2 "correction matmul" with lhsT rows (m, q); then z = r*P.
 * ELU+1, scaled by q:  q*(ELU(z)+1) = min( exp(r*P + ln q), max(P+q, q) )
   -> one ScalarE Exp (psum->sbuf), one VectorE tensor_scalar (add-then-max),
   one VectorE scalar_tensor_tensor min (with a free running sum for the next
   layer's mean via accum_out).
 * r and q come from v (variance) with only the Exp/Ln activation table set:
   lv = ln(v+eps); r = exp(-lv/2); q = exp(+lv/2); ln q = lv/2.
 * Matmuls contract over SBUF partitions, so activations are transposed
   per layer with regular matmuls against an fp16 identity (full PE speed,
   keeps the PE HAM-warm); layer-0 transposed tiles are loaded straight from
   HBM with the DMA xbar transpose (bf16).

Everything on the PE is fp16 (inputs cast host-side), accumulation fp32.
"""

import os
import sys

import numpy as np

sys.path.insert(0, "/opt/trn_rl_repo")

import ml_dtypes

B, F = 262144, 768
N_CORES = 8
ROWS_PER_CORE = B // N_CORES  # 32768
DINS = [768, 512, 256, 128]
DOUTS = [512, 256, 128, 1]
NCH = [d // 128 for d in DINS]  # k-chunks per layer: 6,4,2,1
TILE = 128
EPS = 1e-5

F16 = np.float16

_BUILT = {}
LAST_RESULT = None


_TABLES_PATCHED = False


def _patch_act_tables():
    """Constrain Exp/Ln/Square to the one table set that has all three
    (natural_log_exp_and_others) so bacc's table-load pass doesn't thrash
    between exp_and_others and natural_log on every group (1.3us/load)."""
    global _TABLES_PATCHED
    if _TABLES_PATCHED:
        return
    import concourse.bacc as bacc
    import concourse.hw_specs as hw_specs
    from concourse import mybir

    orig = hw_specs.get_activation_tables
    keep = {
        mybir.ActivationFunctionType.Exp,
        mybir.ActivationFunctionType.Ln,
        mybir.ActivationFunctionType.Square,
    }

    def patched(arch):
        tabs = orig(arch)
        for name in tabs:
            if name != "natural_log_exp_and_others":
                tabs[name] = {f for f in tabs[name] if f not in keep}
        return tabs

    bacc.get_activation_tables = patched
    _TABLES_PATCHED = True


def _build(nrows, G):
    """Build + trace the single-core Bass program for `nrows` rows."""
    import concourse.bacc as bacc
    import concourse.tile as tile
    from concourse import mybir

    _patch_act_tables()

    dt = mybir.dt
    ALU = mybir.AluOpType
    ACTF = mybir.ActivationFunctionType

    ntiles = nrows // TILE
    ngroups = ntiles // G
    assert ntiles % G == 0 and nrows % TILE == 0

    nc = bacc.Bacc("TRN2", target_bir_lowering=False)

    x_d = nc.dram_tensor("x", (nrows, F), dt.float16, kind="ExternalInput")
    w_d = [
        nc.dram_tensor(f"w{i}", (DINS[i], DOUTS[i]), dt.float16, kind="ExternalInput")
        for i in range(4)
    ]
    sc_d = [
        nc.dram_tensor(
            f"scm{i}", (G, 2 * G, DOUTS[i]), dt.float16, kind="ExternalInput"
        )
        for i in range(4)
    ]
    idb_d = nc.dram_tensor("idb", (128, 128), dt.float16, kind="ExternalInput")
    idf_d = nc.dram_tensor("idf", (128, 128), dt.float32, kind="ExternalInput")
    y_d = nc.dram_tensor("y", (nrows,), dt.float32, kind="ExternalOutput")

    xap = x_d[:]
    y_view = y_d[:].rearrange("(a b) -> a b", b=TILE)  # [ntiles, 128]

    GR = G * TILE  # rows per group

    with tile.TileContext(nc) as tc:
        with (
            tc.tile_pool(name="consts", bufs=1) as consts,
            tc.tile_pool(name="xrow", bufs=2 * G) as xrow_p,
            tc.tile_pool(name="xt0", bufs=2) as xt0_p,
            tc.tile_pool(name="gstats", bufs=2) as gst_p,
            tc.tile_pool(name="work", bufs=4) as work_p,
            tc.tile_pool(name="xt", bufs=2 * G + 2) as xt_p,
            tc.tile_pool(name="ybuf", bufs=1) as ybuf_p,
            tc.tile_pool(name="mm_ps", bufs=4, space="PSUM") as mm_ps,
            tc.tile_pool(name="tp_ps", bufs=2, space="PSUM") as tp_ps,
            tc.tile_pool(name="mq_ps", bufs=2, space="PSUM") as mq_ps,
        ):
            # ---- constants ----
            wts = []
            for i in range(4):
                w = consts.tile([128, NCH[i], DOUTS[i]], dt.float16, tag=f"wt{i}")
                nc.sync.dma_start(
                    out=w, in_=w_d[i][:].rearrange("(n p) d -> p n d", p=128)
                )
                wts.append(w)
            scs = []
            for i in range(4):
                s = consts.tile([2 * G, G, DOUTS[i]], dt.float16, tag=f"scm{i}")
                nc.sync.dma_start(out=s, in_=sc_d[i][:].rearrange("t k d -> k t d"))
                scs.append(s)
            idb = consts.tile([128, 128], dt.float16, tag="idb")
            nc.sync.dma_start(out=idb, in_=idb_d[:])
            idf = consts.tile([128, 128], dt.float32, tag="idf")
            nc.sync.dma_start(out=idf, in_=idf_d[:])
            eps_t = consts.tile([128, 1], dt.float32, tag="eps")
            nc.vector.memset(eps_t, EPS)

            y_sb = ybuf_p.tile([128, ntiles], dt.float32, tag="ysb")

            for g in range(ngroups):
                r0 = g * GR
                # ---------- loads ----------
                xrows = []
                for t in range(G):
                    xr = xrow_p.tile([128, F], dt.float16, tag="xr")
                    nc.sync.dma_start(
                        out=xr, in_=xap[r0 + t * TILE : r0 + (t + 1) * TILE, :]
                    )
                    xrows.append(xr)
                xt0 = xt0_p.tile([128, NCH[0], GR], dt.float16, tag="xt0")
                for k in range(NCH[0]):
                    nc.sync.dma_start(
                        out=xt0[:, k, :],
                        in_=xap[r0 : r0 + GR, k * 128 : (k + 1) * 128],
                        transpose=True,
                    )

                # ---------- per-group stats state ----------
                mq_l, r_l, q_l, lnq_l, mqT_l = {}, {}, {}, {}, {}

                def group_stats(l, m_src, v_src):
                    """m_src, v_src: [128, G] f32 APs (mean and variance)."""
                    lv = gst_p.tile([128, G], dt.float32, tag=f"lv{l}")
                    nc.scalar.activation(out=lv, in_=v_src, func=ACTF.Ln, bias=eps_t)
                    r_ = gst_p.tile([128, G], dt.float32, tag=f"r{l}")
                    nc.scalar.activation(out=r_, in_=lv, func=ACTF.Exp, scale=-0.5)
                    q_ = gst_p.tile([128, G], dt.float32, tag=f"q{l}")
                    nc.scalar.activation(out=q_, in_=lv, func=ACTF.Exp, scale=0.5)
                    lnq = gst_p.tile([128, G], dt.float32, tag=f"lnq{l}")
                    nc.vector.tensor_scalar_mul(out=lnq, in0=lv, scalar1=0.5)
                    mq = gst_p.tile([128, G, 2], dt.float16, tag=f"mq{l}")
                    nc.vector.tensor_scalar_mul(out=mq[:, :, 0], in0=m_src, scalar1=1.0)
                    nc.vector.tensor_scalar_mul(out=mq[:, :, 1], in0=q_, scalar1=1.0)
                    # transpose mq -> [2G, 128] for corr-matmul lhsT
                    ps = mq_ps.tile([2 * G, 128], dt.float32, tag="mqps")
                    nc.tensor.matmul(
                        ps, mq[:].rearrange("p a b -> p (a b)"), idb,
                        start=True, stop=True,
                    )
                    mqT = gst_p.tile([2 * G, 128], dt.float16, tag=f"mqT{l}")
                    nc.vector.tensor_copy(out=mqT, in_=ps)
                    mq_l[l], r_l[l], q_l[l], lnq_l[l], mqT_l[l] = mq, r_, q_, lnq, mqT

                # ---------- layer 0 stats (bn_stats over row-major x) ----------
                mv0 = gst_p.tile([128, G, 2], dt.float32, tag="mv0")
                for t in range(G):
                    st = work_p.tile([128, 2, 6], dt.float32, tag="bnst")
                    xr3 = xrows[t][:].rearrange("p (c d) -> p c d", c=2)
                    nc.vector.bn_stats(out=st[:, 0, :], in_=xr3[:, 0, :])
                    nc.vector.bn_stats(out=st[:, 1, :], in_=xr3[:, 1, :])
                    nc.vector.bn_aggr(out=mv0[:, t, :], in_=st)
                group_stats(0, mv0[:, :, 0], mv0[:, :, 1])

                # running sums for layers 1..3 stats
                msum, ssq = {}, {}
                for l in (1, 2, 3):
                    msum[l] = gst_p.tile([128, G], dt.float32, name=f"msum{l}", tag=f"msum{l}")
                    ssq[l] = gst_p.tile([128, G], dt.float32, name=f"ssq{l}", tag=f"ssq{l}")

                xts = {0: [xt0[:, :, t * TILE : (t + 1) * TILE] for t in range(G)]}

                # ---------- layers ----------
                for l in range(4):
                    din, dout, nch = DINS[l], DOUTS[l], NCH[l]
                    if l > 0:
                        ms = gst_p.tile([128, G], dt.float32, tag=f"ms{l}")
                        nc.vector.tensor_scalar_mul(
                            out=ms, in0=msum[l], scalar1=1.0 / din
                        )
                        m2 = gst_p.tile([128, G], dt.float32, tag=f"m2{l}")
                        nc.vector.tensor_mul(m2, ms, ms)
                        v = gst_p.tile([128, G], dt.float32, tag=f"v{l}")
                        nc.vector.scalar_tensor_tensor(
                            out=v,
                            in0=ssq[l],
                            scalar=1.0 / din,
                            in1=m2,
                            op0=ALU.mult,
                            op1=ALU.subtract,
                        )
                        group_stats(l, ms, v)

                    mqT = mqT_l[l]
                    r_, q_, lnq = r_l[l], q_l[l], lnq_l[l]

                    if l == 3:
                        yps = mm_ps.tile([128, G], dt.float32, tag="mm", name="yps")

                    for t in range(G):
                        xT = xts[l][t]  # [128, nch, 128] view
                        # ---- matmul: P = corr + x'' @ Wg.T ----
                        if l < 3:
                            ps = mm_ps.tile([128, dout], dt.float32, tag="mm")
                        else:
                            ps = yps[:, t : t + 1]
                        nc.tensor.matmul(
                            ps, mqT, scs[l][:, t, :],
                            start=True, stop=False,
                        )
                        for k in range(nch):
                            nc.tensor.matmul(
                                ps, xT[:, k, :], wts[l][:, k, :],
                                start=False, stop=(k == nch - 1),
                            )
                        if l == 3:
                            continue

                        # ---- ELU(+1, q-scaled) ----
                        rt = r_[:, t : t + 1]
                        qt = q_[:, t : t + 1]
                        lnqt = lnq[:, t : t + 1]
                        E = work_p.tile([128, dout], dt.float16, tag=f"E{l}")
                        nc.scalar.activation(
                            out=E, in_=ps, func=ACTF.Exp, scale=rt, bias=lnqt
                        )
                        T = work_p.tile([128, dout], dt.float16, tag=f"T{l}")
                        nc.vector.tensor_scalar(
                            out=T,
                            in0=ps,
                            scalar1=qt,
                            scalar2=qt,
                            op0=ALU.add,
                            op1=ALU.max,
                        )
                        xn = work_p.tile([128, dout], dt.float16, tag=f"xn{l}")
                        nc.vector.scalar_tensor_tensor(
                            out=xn,
                            in0=T,
                            scalar=0.0,
                            in1=E,
                            op0=ALU.add,
                            op1=ALU.min,
                            accum_out=msum[l + 1][:, t : t + 1],
                        )
                        # sum of squares for next layer's variance
                        sq = work_p.tile([128, dout], dt.float16, tag=f"sq{l}")
                        nc.scalar.activation(
                            out=sq,
                            in_=xn,
                            func=ACTF.Square,
                            accum_out=ssq[l + 1][:, t : t + 1],
                        )
                        # ---- transpose xn for next layer ----
                        ncho = dout // 128
                        tps = tp_ps.tile([128, ncho, 128], dt.float32, tag="tp")
                        for k in range(ncho):
                            nc.tensor.matmul(
                                tps[:, k, :],
                                xn[:, k * 128 : (k + 1) * 128],
                                idb,
                                start=True,
                                stop=True,
                            )
                        xTn = xt_p.tile([128, ncho, 128], dt.float16, tag=f"xT{l + 1}")
                        nc.vector.tensor_copy(out=xTn, in_=tps)
                        xts.setdefault(l + 1, {})[t] = xTn

                    # ---- layer 3 epilogue: y = r3 * P3 for the whole group ----
                    if l == 3:
                        nc.vector.tensor_mul(y_sb[:, g * G : (g + 1) * G], yps, r_)

            # ---------- output: transpose y_sb [128, ntiles] and store ----------
            nfull = ntiles // 128
            for c in range(nfull):
                ps = tp_ps.tile([128, 128], dt.float32, tag="tp")
                nc.tensor.transpose(ps, y_sb[:, c * 128 : (c + 1) * 128], idf)
                yT = work_p.tile([128, 128], dt.float32, tag="yTs")
                nc.vector.tensor_copy(out=yT, in_=ps)
                nc.sync.dma_start(out=y_view[c * 128 : (c + 1) * 128, :], in_=yT)
            for t in range(nfull * 128, ntiles):
                # small-row builds (sim): strided DMA per tile, slow but correct
                nc.sync.dma_start(out=y_view[t, :], in_=y_sb[:, t])

    nc.compile()
    return nc


def _get_program(nrows, G):
    key = (nrows, G)
    if key not in _BUILT:
        _BUILT[key] = _build(nrows, G)
    return _BUILT[key]


def _fold_weights(inputs, G=8):
    """Host-side folding of LN affine + bias into matmul weights."""
    consts = {}
    for i in range(4):
        g = np.asarray(inputs[f"g{i}"], np.float32)
        b = np.asarray(inputs[f"b{i}"], np.float32)
        W = np.asarray(inputs[f"W{i}"], np.float32)  # [dout, din]
        c = np.asarray(inputs[f"c{i}"], np.float32)
        dout, din = W.shape
        Wg = W * g[None, :]
        c_eff = c + b @ W.T
        s = Wg.sum(axis=1)
        consts[f"w{i}"] = np.ascontiguousarray(Wg.T).astype(F16)  # [din, dout]
        scm = np.zeros((G, 2 * G, dout), np.float32)
        for t in range(G):
            scm[t, 2 * t, :] = -s
            scm[t, 2 * t + 1, :] = c_eff
        consts[f"scm{i}"] = scm.astype(F16)  # [G, 2G, dout]
    consts["idb"] = np.eye(128, dtype=F16)
    consts["idf"] = np.eye(128, dtype=np.float32)
    return consts


def _install_ntff_shim():
    """bass_utils' trace path needs antenv.axon_hooks, absent in this image.
    Recreate it with the ctypes hook from trn_agent_boot (same mechanism)."""
    import types

    try:
        from antenv.axon_hooks import get_axon_ntff_profile_hook  # noqa: F401

        return
    except ImportError:
        pass
    try:
        import antenv
        from trn_agent_boot.trn_boot import _ntff_profile_via_ctypes

        hook = _ntff_profile_via_ctypes("/opt/axon/libaxon_pjrt.so")
        mod = types.ModuleType("antenv.axon_hooks")
        mod._hook = hook
        mod.get_axon_ntff_profile_hook = lambda: mod._hook
        mod.set_axon_ntff_profile_hook = lambda h: setattr(mod, "_hook", h)
        sys.modules["antenv.axon_hooks"] = mod
        antenv.axon_hooks = mod
    except Exception as e:  # tracing is best-effort
        print(f"ntff shim failed: {e}")


def kernel(**inputs):
    from concourse.bass_utils import run_bass_kernel_spmd

    x = np.asarray(inputs["input"])
    assert x.shape == (B, F)
    consts = _fold_weights(inputs)
    x_bf = x.astype(F16)

    nc = _get_program(ROWS_PER_CORE, 8)
    in_maps = []
    for cid in range(N_CORES):
        shard = x_bf[cid * ROWS_PER_CORE : (cid + 1) * ROWS_PER_CORE]
        in_maps.append({"x": np.ascontiguousarray(shard), **consts})
    trace = bool(os.environ.get("KERNEL_TRACE"))
    if trace:
        _install_ntff_shim()
    res = run_bass_kernel_spmd(
        nc, in_maps, core_ids=list(range(N_CORES)), trace=trace
    )
    global LAST_RESULT
    LAST_RESULT = res
    out = np.concatenate([np.asarray(res.results[c]["y"]) for c in range(N_CORES)])
    return out.astype(np.float32)


# revision 14
# speedup vs baseline: 1.2296x; 1.1510x over previous
"""Trainium2 Bass kernel for nn_DNN_65944927863183 (dense_mlp).

Network (per row of input [B=262144, F=768]):
    4 blocks of  x = LN(x; g,b) @ W.T + c ; ELU between blocks (not after last)
    sizes 768 -> 512 -> 256 -> 128 -> 1, output squeezed to [B].

Strategy
--------
Pure data parallel over 8 NeuronCores: each core gets B/8 = 32768 contiguous
rows, weights replicated. Inside a core, rows are processed in 128-row tiles
(rows on SBUF partitions), 16 tiles per "group" for batching small ops.

Math tricks (all exact up to float rounding):
 * LN affine (g, b) and linear bias c are folded into the weights on the host:
   y = LNhat(x) @ (W*g).T + (b@W.T + c), LNhat = (x-m)/sqrt(var+eps).
 * LN is invariant to per-row affine of its input. So each layer stores
   x' = q * (ELU(z) + 1)   (q = std of this layer's input, per row)
   instead of ELU(z); the next LN removes the per-row scale/shift exactly
   (the 1e-5 epsilon is scaled by q^2 in later layers; |error| ~ 1e-4 rel).
 * Normalization is folded into the matmul: with stored input x'' (stats m,
   q = sqrt(var+eps), r = 1/q), the pre-activation in PSUM is
       P = x''@Wg.T + m*(-s) + q*c     (s_j = sum_f Wg[j,f]),
   via one K=2 "correction matmul" with lhsT rows (m, q); then z = r*P.
 * ELU+1, scaled by q:  q*(ELU(z)+1) = min( exp(r*P + ln q), max(P+q, q) )
   -> one ScalarE Exp (psum->sbuf), one VectorE tensor_scalar (add-then-max),
   one VectorE scalar_tensor_tensor min (with a free running sum for the next
   layer's mean via accum_out).
 * r and q come from v (variance) with only the Exp/Ln activation table set:
   lv = ln(v+eps); r = exp(-lv/2); q = exp(+lv/2); ln q = lv/2.
 * Matmuls contract over SBUF partitions, so activations are transposed
   per layer with regular matmuls against an fp16 identity (full PE speed,
   keeps the PE HAM-warm); layer-0 transposed tiles are loaded straight from
   HBM with the DMA xbar transpose (bf16).

Everything on the PE is fp16 (inputs cast host-side), accumulation fp32.
"""

import os
import sys

import numpy as np

sys.path.insert(0, "/opt/trn_rl_repo")

import ml_dtypes

B, F = 262144, 768
N_CORES = 8
ROWS_PER_CORE = B // N_CORES  # 32768
DINS = [768, 512, 256, 128]
DOUTS = [512, 256, 128, 1]
NCH = [d // 128 for d in DINS]  # k-chunks per layer: 6,4,2,1
TILE = 128
EPS = 1e-5

F16 = np.float16

_BUILT = {}
LAST_RESULT = None


_TABLES_PATCHED = False


def _patch_act_tables():
    """Constrain Exp/Ln/Square to the one table set that has all three
    (natural_log_exp_and_others) so bacc's table-load pass doesn't thrash
    between exp_and_others and natural_log on every group (1.3us/load)."""
    global _TABLES_PATCHED
    if _TABLES_PATCHED:
        return
    import concourse.bacc as bacc
    import concourse.hw_specs as hw_specs
    from concourse import mybir

    orig = hw_specs.get_activation_tables
    keep = {
        mybir.ActivationFunctionType.Exp,
        mybir.ActivationFunctionType.Ln,
        mybir.ActivationFunctionType.Square,
    }

    def patched(arch):
        tabs = orig(arch)
        for name in tabs:
            if name != "natural_log_exp_and_others":
                tabs[name] = {f for f in tabs[name] if f not in keep}
        return tabs

    bacc.get_activation_tables = patched
    _TABLES_PATCHED = True


def _build(nrows, G):
    """Build + trace the single-core Bass program for `nrows` rows."""
    import concourse.bacc as bacc
    import concourse.tile as tile
    from concourse import mybir

    _patch_act_tables()

    dt = mybir.dt
    ALU = mybir.AluOpType
    ACTF = mybir.ActivationFunctionType

    ntiles = nrows // TILE
    ngroups = ntiles // G
    assert ntiles % G == 0 and nrows % TILE == 0

    nc = bacc.Bacc("TRN2", target_bir_lowering=False)

    x_d = nc.dram_tensor("x", (nrows, F), dt.float16, kind="ExternalInput")
    w_d = [
        nc.dram_tensor(f"w{i}", (DINS[i], DOUTS[i]), dt.float16, kind="ExternalInput")
        for i in range(4)
    ]
    sc_d = [
        nc.dram_tensor(
            f"scm{i}", (G, 2 * G, DOUTS[i]), dt.float16, kind="ExternalInput"
        )
        for i in range(4)
    ]
    idb_d = nc.dram_tensor("idb", (128, 128), dt.float16, kind="ExternalInput")
    idf_d = nc.dram_tensor("idf", (128, 128), dt.float32, kind="ExternalInput")
    y_d = nc.dram_tensor("y", (nrows,), dt.float32, kind="ExternalOutput")

    xap = x_d[:]
    y_view = y_d[:].rearrange("(a b) -> a b", b=TILE)  # [ntiles, 128]

    GR = G * TILE  # rows per group

    with tile.TileContext(nc) as tc:
        with (
            tc.tile_pool(name="consts", bufs=1) as consts,
            tc.tile_pool(name="xrow", bufs=2 * G + 4) as xrow_p,
            tc.tile_pool(name="xt0", bufs=3) as xt0_p,
            tc.tile_pool(name="gstats", bufs=3) as gst_p,
            tc.tile_pool(name="work", bufs=6) as work_p,
            tc.tile_pool(name="xt", bufs=2 * G + 4) as xt_p,
            tc.tile_pool(name="ybuf", bufs=1) as ybuf_p,
            tc.tile_pool(name="mm_ps", bufs=4, space="PSUM") as mm_ps,
            tc.tile_pool(name="tp_ps", bufs=2, space="PSUM") as tp_ps,
            tc.tile_pool(name="mq_ps", bufs=2, space="PSUM") as mq_ps,
        ):
            # ---- constants ----
            wts = []
            for i in range(4):
                w = consts.tile([128, NCH[i], DOUTS[i]], dt.float16, tag=f"wt{i}")
                nc.sync.dma_start(
                    out=w, in_=w_d[i][:].rearrange("(n p) d -> p n d", p=128)
                )
                wts.append(w)
            scs = []
            for i in range(4):
                s = consts.tile([2 * G, G, DOUTS[i]], dt.float16, tag=f"scm{i}")
                nc.sync.dma_start(out=s, in_=sc_d[i][:].rearrange("t k d -> k t d"))
                scs.append(s)
            idb = consts.tile([128, 128], dt.float16, tag="idb")
            nc.sync.dma_start(out=idb, in_=idb_d[:])
            idf = consts.tile([128, 128], dt.float32, tag="idf")
            nc.sync.dma_start(out=idf, in_=idf_d[:])
            eps_t = consts.tile([128, 1], dt.float32, tag="eps")
            nc.vector.memset(eps_t, EPS)

            y_sb = ybuf_p.tile([128, ntiles], dt.float32, tag="ysb")

            def group_stats(l, m_src, v_src, S):
                """m_src, v_src: [128, G] f32 APs (mean and variance)."""
                lv = gst_p.tile([128, G], dt.float32, tag=f"lv{l}", name="lv")
                nc.scalar.activation(out=lv, in_=v_src, func=ACTF.Ln, bias=eps_t)
                r_ = gst_p.tile([128, G], dt.float32, tag=f"r{l}", name="r_")
                nc.scalar.activation(out=r_, in_=lv, func=ACTF.Exp, scale=-0.5)
                q_ = gst_p.tile([128, G], dt.float32, tag=f"q{l}", name="q_")
                nc.scalar.activation(out=q_, in_=lv, func=ACTF.Exp, scale=0.5)
                lnq = gst_p.tile([128, G], dt.float32, tag=f"lnq{l}", name="lnq")
                nc.vector.tensor_scalar_mul(out=lnq, in0=lv, scalar1=0.5)
                mq = gst_p.tile([128, G, 2], dt.float16, tag=f"mq{l}", name="mq")
                nc.vector.tensor_scalar_mul(out=mq[:, :, 0], in0=m_src, scalar1=1.0)
                nc.vector.tensor_scalar_mul(out=mq[:, :, 1], in0=q_, scalar1=1.0)
                # transpose mq -> [2G, 128] for corr-matmul lhsT
                ps = mq_ps.tile([2 * G, 128], dt.float32, tag="mqps", name="mqps")
                nc.tensor.matmul(
                    ps, mq[:].rearrange("p a b -> p (a b)"), idb,
                    start=True, stop=True,
                )
                mqT = gst_p.tile([2 * G, 128], dt.float16, tag=f"mqT{l}", name="mqT")
                nc.vector.tensor_copy(out=mqT, in_=ps)
                S["stats"][l] = (r_, q_, lnq, mqT)

            def emit_copies(g, S):
                r0 = g * GR
                xrows = []
                for t in range(G):
                    xr = xrow_p.tile([128, F], dt.float16, tag="xr", name="xr")
                    nc.sync.dma_start(
                        out=xr, in_=xap[r0 + t * TILE : r0 + (t + 1) * TILE, :]
                    )
                    xrows.append(xr)
                S["xrows"] = xrows

            def emit_transposes(g, S):
                r0 = g * GR
                xt0 = xt0_p.tile([128, NCH[0], GR], dt.float16, tag="xt0", name="xt0")
                for k in range(NCH[0]):
                    nc.sync.dma_start(
                        out=xt0[:, k, :],
                        in_=xap[r0 : r0 + GR, k * 128 : (k + 1) * 128],
                        transpose=True,
                    )
                S["xts"] = {0: [xt0[:, :, t * TILE : (t + 1) * TILE] for t in range(G)]}

            def emit_stats0(g, S):
                mv0 = gst_p.tile([128, G, 2], dt.float32, tag="mv0", name="mv0")
                for t in range(G):
                    st = work_p.tile([128, 2, 6], dt.float32, tag="bnst", name="st")
                    xr3 = S["xrows"][t][:].rearrange("p (c d) -> p c d", c=2)
                    nc.vector.bn_stats(out=st[:, 0, :], in_=xr3[:, 0, :])
                    nc.vector.bn_stats(out=st[:, 1, :], in_=xr3[:, 1, :])
                    nc.vector.bn_aggr(out=mv0[:, t, :], in_=st)
                group_stats(0, mv0[:, :, 0], mv0[:, :, 1], S)
                for l in (1, 2, 3):
                    S["msum"][l] = gst_p.tile(
                        [128, G], dt.float32, name="msum", tag=f"msum{l}"
                    )
                    S["ssq"][l] = gst_p.tile(
                        [128, G], dt.float32, name="ssq", tag=f"ssq{l}"
                    )

            def emit_layer(g, l, S):
                din, dout, nch = DINS[l], DOUTS[l], NCH[l]
                msum, ssq = S["msum"], S["ssq"]
                if l > 0:
                    ms = gst_p.tile([128, G], dt.float32, tag=f"ms{l}", name="ms")
                    nc.vector.tensor_scalar_mul(out=ms, in0=msum[l], scalar1=1.0 / din)
                    m2 = gst_p.tile([128, G], dt.float32, tag=f"m2{l}", name="m2")
                    nc.vector.tensor_mul(m2, ms, ms)
                    v = gst_p.tile([128, G], dt.float32, tag=f"v{l}", name="v")
                    nc.vector.scalar_tensor_tensor(
                        out=v,
                        in0=ssq[l],
                        scalar=1.0 / din,
                        in1=m2,
                        op0=ALU.mult,
                        op1=ALU.subtract,
                    )
                    group_stats(l, ms, v, S)

                r_, q_, lnq, mqT = S["stats"][l]

                if l == 3:
                    yps = mm_ps.tile([128, G], dt.float32, tag="mm", name="yps")

                for t in range(G):
                    xT = S["xts"][l][t]  # [128, nch, 128] view
                    # ---- matmul: P = corr + x'' @ Wg.T ----
                    if l < 3:
                        ps = mm_ps.tile([128, dout], dt.float32, tag="mm", name="ps")
                    else:
                        ps = yps[:, t : t + 1]
                    nc.tensor.matmul(ps, mqT, scs[l][:, t, :], start=True, stop=False)
                    for k in range(nch):
                        nc.tensor.matmul(
                            ps, xT[:, k, :], wts[l][:, k, :],
                            start=False, stop=(k == nch - 1),
                        )
                    if l == 3:
                        continue

                    # ---- ELU(+1, q-scaled):
                    #      xn = min(exp(r*P + lnq), max(P, 0) + q) ----
                    rt = r_[:, t : t + 1]
                    qt = q_[:, t : t + 1]
                    lnqt = lnq[:, t : t + 1]
                    E = work_p.tile([128, dout], dt.float16, tag=f"E{l}", name="E")
                    nc.scalar.activation(
                        out=E, in_=ps, func=ACTF.Exp, scale=rt, bias=lnqt
                    )
                    T = work_p.tile([128, dout], dt.float16, tag=f"T{l}", name="T")
                    if l == 0:
                        # line branch on ScalarE (DVE is the busiest engine)
                        nc.scalar.activation(out=T, in_=ps, func=ACTF.Relu)
                        qadd = qt
                    else:
                        nc.vector.tensor_scalar(
                            out=T, in0=ps, scalar1=qt, scalar2=qt,
                            op0=ALU.add, op1=ALU.max,
                        )
                        qadd = 0.0
                    xn = work_p.tile([128, dout], dt.float16, tag=f"xn{l}", name="xn")
                    nc.vector.scalar_tensor_tensor(
                        out=xn,
                        in0=T,
                        scalar=qadd,
                        in1=E,
                        op0=ALU.add,
                        op1=ALU.min,
                        accum_out=msum[l + 1][:, t : t + 1],
                    )
                    # sum of squares for next layer's variance
                    sq = work_p.tile([128, dout], dt.float16, tag=f"sq{l}", name="sq")
                    nc.scalar.activation(
                        out=sq,
                        in_=xn,
                        func=ACTF.Square,
                        accum_out=ssq[l + 1][:, t : t + 1],
                    )
                    # ---- transpose xn for next layer ----
                    ncho = dout // 128
                    tps = tp_ps.tile([128, ncho, 128], dt.float32, tag="tp", name="tps")
                    for k in range(ncho):
                        nc.tensor.matmul(
                            tps[:, k, :],
                            xn[:, k * 128 : (k + 1) * 128],
                            idb,
                            start=True,
                            stop=True,
                        )
                    xTn = xt_p.tile(
                        [128, ncho, 128], dt.float16, tag=f"xT{l + 1}", name="xTn"
                    )
                    nc.vector.tensor_copy(out=xTn, in_=tps)
                    S["xts"].setdefault(l + 1, {})[t] = xTn

                # ---- layer 3 epilogue: y = r3 * P3 for the whole group ----
                if l == 3:
                    nc.vector.tensor_mul(y_sb[:, g * G : (g + 1) * G], yps, r_)

            assert ngroups % 2 == 0
            for g0 in range(0, ngroups, 2):
                pair = (g0, g0 + 1)
                states = {g: {"stats": {}, "msum": {}, "ssq": {}} for g in pair}
                for g in pair:
                    emit_copies(g, states[g])
                for g in pair:
                    emit_transposes(g, states[g])
                for g in pair:
                    emit_stats0(g, states[g])
                for l in range(4):
                    for g in pair:
                        emit_layer(g, l, states[g])

            # ---------- output: transpose y_sb [128, ntiles] and store ----------
            nfull = ntiles // 128
            for c in range(nfull):
                ps = tp_ps.tile([128, 128], dt.float32, tag="tp")
                nc.tensor.transpose(ps, y_sb[:, c * 128 : (c + 1) * 128], idf)
                yT = work_p.tile([128, 128], dt.float32, tag="yTs")
                nc.vector.tensor_copy(out=yT, in_=ps)
                nc.sync.dma_start(out=y_view[c * 128 : (c + 1) * 128, :], in_=yT)
            for t in range(nfull * 128, ntiles):
                # small-row builds (sim): strided DMA per tile, slow but correct
                nc.sync.dma_start(out=y_view[t, :], in_=y_sb[:, t])

    nc.compile()
    return nc


def _get_program(nrows, G):
    key = (nrows, G)
    if key not in _BUILT:
        _BUILT[key] = _build(nrows, G)
    return _BUILT[key]


def _fold_weights(inputs, G=8):
    """Host-side folding of LN affine + bias into matmul weights."""
    consts = {}
    for i in range(4):
        g = np.asarray(inputs[f"g{i}"], np.float32)
        b = np.asarray(inputs[f"b{i}"], np.float32)
        W = np.asarray(inputs[f"W{i}"], np.float32)  # [dout, din]
        c = np.asarray(inputs[f"c{i}"], np.float32)
        dout, din = W.shape
        Wg = W * g[None, :]
        c_eff = c + b @ W.T
        s = Wg.sum(axis=1)
        consts[f"w{i}"] = np.ascontiguousarray(Wg.T).astype(F16)  # [din, dout]
        scm = np.zeros((G, 2 * G, dout), np.float32)
        for t in range(G):
            scm[t, 2 * t, :] = -s
            scm[t, 2 * t + 1, :] = c_eff
        consts[f"scm{i}"] = scm.astype(F16)  # [G, 2G, dout]
    consts["idb"] = np.eye(128, dtype=F16)
    consts["idf"] = np.eye(128, dtype=np.float32)
    return consts


def _install_ntff_shim():
    """bass_utils' trace path needs antenv.axon_hooks, absent in this image.
    Recreate it with the ctypes hook from trn_agent_boot (same mechanism)."""
    import types

    try:
        from antenv.axon_hooks import get_axon_ntff_profile_hook  # noqa: F401

        return
    except ImportError:
        pass
    try:
        import antenv
        from trn_agent_boot.trn_boot import _ntff_profile_via_ctypes

        hook = _ntff_profile_via_ctypes("/opt/axon/libaxon_pjrt.so")
        mod = types.ModuleType("antenv.axon_hooks")
        mod._hook = hook
        mod.get_axon_ntff_profile_hook = lambda: mod._hook
        mod.set_axon_ntff_profile_hook = lambda h: setattr(mod, "_hook", h)
        sys.modules["antenv.axon_hooks"] = mod
        antenv.axon_hooks = mod
    except Exception as e:  # tracing is best-effort
        print(f"ntff shim failed: {e}")


def kernel(**inputs):
    from concourse.bass_utils import run_bass_kernel_spmd

    x = np.asarray(inputs["input"])
    assert x.shape == (B, F)
    consts = _fold_weights(inputs)
    x_bf = x.astype(F16)

    nc = _get_program(ROWS_PER_CORE, 8)
    in_maps = []
    for cid in range(N_CORES):
        shard = x_bf[cid * ROWS_PER_CORE : (cid + 1) * ROWS_PER_CORE]
        in_maps.append({"x": np.ascontiguousarray(shard), **consts})
    trace = bool(os.environ.get("KERNEL_TRACE"))
    if trace:
        _install_ntff_shim()
    res = run_bass_kernel_spmd(
        nc, in_maps, core_ids=list(range(N_CORES)), trace=trace
    )
    global LAST_RESULT
    LAST_RESULT = res
    out = np.concatenate([np.asarray(res.results[c]["y"]) for c in range(N_CORES)])
    return out.astype(np.float32)
